# revision 17
# baseline (speedup 1.0000x reference)
"""Adaptive-threshold LIF neuron recurrence (SNN) on 8 Trainium2 NeuronCores.

Strategy
--------
The recurrence is data-parallel over the 262144 neurons except for a scalar
firing-rate EMA that couples every neuron at every timestep (the spike MEAN
feeds the next step's threshold).  A per-step AllReduce would cost ~10us x
128 steps, so instead:

  host:   estimate the threshold-offset sequence C0[t] from a neuron
          subsample (cheap numpy sim)
  device: ONE data-parallel pass with C0 per core, recording per-step
          spike/membrane/adaptation partial sums (exact f32 integers for
          spikes) AND each neuron's minimum |u - C0| margin over time
  host:   compute the exact global EMA chain from the (integer) spike sums
          -> corrected C[t]; every neuron whose min margin exceeds the C
          correction provably has an identical trajectory; the few
          (~10^2-10^3) neurons inside the margin are re-simulated in numpy
          with bit-exact replicas of the device arithmetic, iterating the
          tiny fixed point until the C chain is stable; spikes and sums are
          patched accordingly.

Spike sums are sums of 0/1 values, so every f32 reduction of them is exact
(integers < 2^24), which makes the host EMA chain exact.  A 2-pass fully
on-device variant (P1 -> AllReduce -> P2) is kept as a fallback.

Per-core layout: features sharded 8 ways (1024 per core), time-major tiles
x[tb, p, g*256+j] = input[t = tb*G+g, neuron n = p*256+j], n = b*1024+f.

Per-step device math (one [128,256] f32 tile per core per step), with
G := 0.1*gamma*adapt pre-scaled so the threshold compare needs no extra op:
  syn   = (syn * beta) + x_t                 (GPSIMD: 2x tensor_tensor)
  mem   = (mem * alpha) + syn                (V: stt, accum -> mem sums)
  u     = (mem * k1) - G                     (V: stt)   [u == mem - 0.1*adapt]
  tmp   = k2 * mem                           (ACT copy)
  G     = (G * gamma) + tmp                  (V: stt, accum -> G sums)
  spike = u >= C_t                           (V: tensor_scalar, accum -> sums)
  e     = relu(u - C_t)                      (ACT relu, bias = -C)
  mem   = spike ? e : mem                    (V: copy_predicated)
  ab    = |u - C_t|                          (ACT abs, bias = -C)
  sg    = sign(ab - theta0)                  (ACT sign: -1/0 if near margin)
  asum  = asum + sg                          (GPSIMD tensor_tensor add)
(neuron is provably unaffected by C refinements iff asum == T)
"""

import math
import sys
import time

if "/opt/trn_rl_repo" not in sys.path:
    sys.path.insert(0, "/opt/trn_rl_repo")

import numpy as np

# ---------------------------------------------------------------- constants
B, F, T = 32, 8192, 128
N_CORES = 8
F_LOC = F // N_CORES            # 1024 features per core
N_LOC = B * F_LOC               # 32768 neurons per core
N_GLOB = B * F                  # 262144 neurons total
P = 128                         # SBUF partitions
FD = N_LOC // P                 # 256 free-dim elems per step tile
G = 16                          # timesteps per DMA group
TB = T // G                     # 8 groups

# exact f32 constants as produced by the jax reference (verified bitwise)
ALPHA = np.uint32(0x3F7383C5).view(np.float32)  # exp(-.001/.02)
BETA = np.uint32(0x3F519857).view(np.float32)   # exp(-.001/.005)
GAMMA = np.uint32(0x3F7D73E8).view(np.float32)  # exp(-.001/.1)
C1G = np.uint32(0x3C230600).view(np.float32)    # 1 - gamma
ADAPT_STRENGTH = np.float32(0.1)
HOMEO_RATE = np.float32(0.01)
THRESHOLD_BASE = np.float32(1.0)

K1 = np.float32(1.0 - 0.1 * float(C1G))            # 1 - 0.1*(1-gamma)
K2 = np.float32(0.1 * float(GAMMA) * float(C1G))   # 0.1*gamma*(1-gamma)
# G update rewritten in terms of u (so no engine reads mem after the reset):
#   G_t = (gamma + k2/k1)*G_{t-1} + (k2/k1)*u_t
KK = np.float32(float(K2) / float(K1))             # k2/k1
G2 = np.float32(float(GAMMA) + float(K2) / float(K1))

N_PASSES = 1          # 1 = single pass + host margin correction (default)
MARGIN_CAP = 40000    # fall back to a device re-run above this many neurons
THETA0 = 3e-4         # margin radius tracked on device (abs units of u)

_BUILD_CACHE = {}


# ------------------------------------------------------------- device build
def _build_kernel(n_passes=N_PASSES, base=1.0, bias=0.0):
    key = (n_passes, float(base), float(bias))
    if key in _BUILD_CACHE:
        return _BUILD_CACHE[key]

    import concourse.bacc as bacc
    import concourse.mybir as mybir
    from concourse import tile

    DT = mybir.dt.float32
    AF = mybir.ActivationFunctionType
    OP = mybir.AluOpType
    I32 = mybir.dt.int32

    nc = bacc.Bacc(None, target_bir_lowering=False, debug=False,
                   num_devices=N_CORES)

    x_in = nc.dram_tensor("x", [TB, P, G * FD], DT, kind="ExternalInput")
    c0_in = nc.dram_tensor("c0", [1, T], DT, kind="ExternalInput")

    spk_out = nc.dram_tensor("spk", [TB, P, G * FD], DT, kind="ExternalOutput")
    acc_out = nc.dram_tensor("acc", [3, P, T], DT, kind="ExternalOutput")
    crow_out = nc.dram_tensor("crow", [max(n_passes, 2), T], DT,
                              kind="ExternalOutput")
    xtra_out = nc.dram_tensor("xtra", [1, 8], DT, kind="ExternalOutput")
    mm_out = nc.dram_tensor("mm", [P, FD], DT, kind="ExternalOutput")

    with tile.TileContext(nc) as tc:
        with (
            tc.tile_pool(name="state", bufs=1) as st,
            tc.tile_pool(name="xload", bufs=3) as xl,
            tc.tile_pool(name="sout", bufs=2) as so,
            tc.tile_pool(name="psum", bufs=2, space="PSUM") as ps,
            tc.tile_pool(name="dram", bufs=1, space="DRAM") as dram,
        ):
            mem = st.tile([P, FD], DT, tag="mem")
            gst = st.tile([P, FD], DT, tag="gst")
            mm = st.tile([P, FD], DT, tag="mm")
            spk_s = st.tile([P, FD], DT, tag="spk_s")
            ones = st.tile([P, 1], DT, tag="ones")
            nc.vector.memset(ones[:], 1.0)
            betat = st.tile([P, FD], DT, tag="betat")
            nc.gpsimd.memset(betat[:], float(BETA))
            nth0 = st.tile([P, 1], DT, tag="nth0")
            nc.vector.memset(nth0[:], -float(THETA0))
            track_margin = n_passes == 1
            if track_margin:
                nc.gpsimd.memset(mm[:], 0.0)

            # per-pass C matrices ([128, T] broadcast across partitions):
            # positive C for the spike compare (tensor_scalar is_ge),
            # negated C as the ACT Relu/Abs bias
            cmat = [
                st.tile([P, T], DT, tag=f"cmat{p}", name=f"cmat{p}")
                for p in range(n_passes)
            ]
            nmat = [
                st.tile([P, T], DT, tag=f"nmat{p}", name=f"nmat{p}")
                for p in range(n_passes)
            ]
            c0row = st.tile([1, T], DT, tag="c0row")
            nc.sync.dma_start(c0row[:], c0_in[:, :])
            nc0row = st.tile([1, T], DT, tag="nc0row")
            nc.vector.tensor_scalar(nc0row[:], c0row[:], -1.0, None, OP.mult)
            nc.gpsimd.partition_broadcast(cmat[0][:], c0row[:], channels=P)
            nc.gpsimd.partition_broadcast(nmat[0][:], nc0row[:], channels=P)

            # accumulators: [128, T] per quantity (last pass overwrites)
            acc_spk = st.tile([P, T], DT, tag="acc_spk")
            acc_mem = st.tile([P, T], DT, tag="acc_mem")
            acc_g = st.tile([P, T], DT, tag="acc_g")

            # constant row of 0.99 for the ema scan (2-pass mode)
            row99 = st.tile([1, T], DT, tag="row99")
            nc.vector.memset(row99[:], 0.99)

            for p_i in range(n_passes):
                last = p_i == n_passes - 1
                C = cmat[p_i]
                NC = nmat[p_i]
                syn = st.tile([P, FD], DT, tag="syn", name="syn0", bufs=2)
                nc.gpsimd.memset(syn[:], 0.0)
                nc.vector.memset(mem[:], 0.0)
                nc.vector.memset(gst[:], 0.0)

                for tb in range(TB):
                    xt = xl.tile([P, G * FD], DT, tag="xt")
                    nc.sync.dma_start(xt[:], x_in[tb, :, :])
                    if last:
                        ot = so.tile([P, G * FD], DT, tag="ot")
                    for g in range(G):
                        t = tb * G + g
                        xs = xt[:, g * FD:(g + 1) * FD]
                        u = st.tile([P, FD], DT, tag="u", name="u", bufs=2)
                        e = st.tile([P, FD], DT, tag="e", name="e", bufs=2)
                        tmp = st.tile([P, FD], DT, tag="tmp", name="tmp",
                                      bufs=2)
                        ab = st.tile([P, FD], DT, tag="ab", name="ab",
                                     bufs=2)
                        sg = st.tile([P, FD], DT, tag="sg", name="sg",
                                     bufs=2)
                        # syn = beta*syn + x  (GPSIMD, double-buffered so
                        # the t+1 update never waits on V's read of syn_t)
                        syn_new = st.tile([P, FD], DT, tag="syn",
                                          name="syn", bufs=2)
                        nc.gpsimd.tensor_tensor(syn_new[:], syn[:], betat[:],
                                                OP.mult)
                        nc.gpsimd.tensor_tensor(syn_new[:], syn_new[:], xs,
                                                OP.add)
                        syn = syn_new
                        nc.vector.scalar_tensor_tensor(
                            mem[:], mem[:], float(ALPHA), syn[:],
                            OP.mult, OP.add,
                            accum_out=acc_mem[:, t:t + 1] if last else None,
                        )
                        nc.vector.scalar_tensor_tensor(
                            u[:], mem[:], float(K1), gst[:],
                            OP.mult, OP.subtract,
                        )
                        # reset value e = relu(u + (-C)) on the ACT engine
                        # (emitted early: it is on the critical V loop)
                        nc.scalar.activation(
                            e[:], u[:], AF.Relu, bias=NC[:, t:t + 1],
                            scale=1.0,
                        )
                        if last:
                            sp_ap = ot[:, g * FD:(g + 1) * FD]
                        else:
                            sp_ap = spk_s[:]
                        # spike = (u >= C), accum(reduce add) -> step sums
                        nc.vector.tensor_scalar(
                            sp_ap, u[:], C[:, t:t + 1], 0.0,
                            OP.is_ge, OP.add,
                            accum_out=acc_spk[:, t:t + 1],
                        )
                        nc.vector.copy_predicated(
                            mem[:], sp_ap.bitcast(I32), e[:]
                        )
                        nc.scalar.activation(tmp[:], u[:], AF.Copy,
                                             scale=float(KK))
                        nc.vector.scalar_tensor_tensor(
                            gst[:], gst[:], float(G2), tmp[:],
                            OP.mult, OP.add,
                            accum_out=acc_g[:, t:t + 1] if last else None,
                        )
                        if track_margin:
                            nc.scalar.activation(
                                ab[:], u[:], AF.Abs, bias=NC[:, t:t + 1],
                                scale=1.0,
                            )
                            nc.scalar.activation(
                                sg[:], ab[:], AF.Sign, bias=nth0[:, 0:1],
                                scale=1.0,
                            )
                            nc.gpsimd.tensor_tensor(mm[:], mm[:], sg[:],
                                                    OP.add)
                    if last:
                        nc.sync.dma_start(spk_out[tb, :, :], ot[:])

                # ---- inter-pass: global spike sums -> next C (2-pass mode)
                if not last:
                    pt = ps.tile([1, T], DT, tag="pt")
                    nc.tensor.matmul(pt[:], ones[:], acc_spk[:],
                                     start=True, stop=True)
                    srow = st.tile([1, T], DT, tag=f"srow{p_i}",
                                   name=f"srow{p_i}")
                    nc.scalar.copy(srow[:], pt[:])
                    cc_in = dram.tile([1, T], DT, name=f"ccin{p_i}")
                    cc_out = dram.tile([1, T], DT, name=f"ccout{p_i}")
                    nc.sync.dma_start(cc_in[:], srow[:])
                    nc.gpsimd.collective_compute(
                        "AllReduce", OP.add,
                        replica_groups=[list(range(N_CORES))],
                        ins=[cc_in.opt()], outs=[cc_out.opt()],
                    )
                    sglob = st.tile([1, T], DT, tag=f"sglob{p_i}",
                                    name=f"sglob{p_i}")
                    nc.sync.dma_start(sglob[:], cc_out[:])
                    # ema chain: msc = sums * (0.01/N_GLOB); scan; homeo; C
                    mean_scale = np.float32(
                        np.float32(0.01) * np.float32(1.0 / N_GLOB))
                    msc = st.tile([1, T], DT, tag=f"msc{p_i}",
                                  name=f"msc{p_i}")
                    nc.vector.tensor_scalar(
                        msc[:], sglob[:], float(mean_scale), None, OP.mult)
                    ema = st.tile([1, T], DT, tag=f"ema{p_i}",
                                  name=f"ema{p_i}")
                    nc.vector.tensor_tensor_scan(
                        ema[:], row99[:], msc[:], 0.0, OP.mult, OP.add)
                    hm = st.tile([1, T], DT, tag=f"hm{p_i}", name=f"hm{p_i}")
                    nc.vector.tensor_scalar(
                        hm[:], ema[:], float(HOMEO_RATE), 0.0,
                        OP.subtract, OP.max)
                    cn = st.tile([1, T], DT, tag=f"cn{p_i}", name=f"cn{p_i}")
                    # C[0] is pass-independent (ema starts at 0)
                    nc.vector.tensor_copy(cn[:, 0:1], c0row[:, 0:1])
                    nc.vector.tensor_scalar(
                        cn[:, 1:T], hm[:, 0:T - 1], float(ADAPT_STRENGTH),
                        float(base), OP.mult, OP.add)
                    if float(bias) != 0.0:
                        nc.vector.tensor_scalar(
                            cn[:, 1:T], cn[:, 1:T], float(bias), None,
                            OP.add)
                    ncn = st.tile([1, T], DT, tag=f"ncn{p_i}",
                                  name=f"ncn{p_i}")
                    nc.vector.tensor_scalar(
                        ncn[:], cn[:], -1.0, None, OP.mult)
                    nc.gpsimd.partition_broadcast(
                        cmat[p_i + 1][:], cn[:], channels=P)
                    nc.gpsimd.partition_broadcast(
                        nmat[p_i + 1][:], ncn[:], channels=P)
                    nc.sync.dma_start(crow_out[p_i + 1:p_i + 2, :], cn[:])

            # ---- final extras: sum of post-reset mem at t=T-1
            memsum = st.tile([P, 1], DT, tag="memsum")
            nc.vector.reduce_sum(memsum[:], mem[:], axis=mybir.AxisListType.X)
            ptx = ps.tile([1, 1], DT, tag="ptx")
            nc.tensor.matmul(ptx[:], ones[:], memsum[:], start=True, stop=True)
            xrow = st.tile([1, 8], DT, tag="xrow")
            nc.vector.memset(xrow[:], 0.0)
            nc.scalar.copy(xrow[:, 0:1], ptx[:])
            nc.sync.dma_start(xtra_out[:, :], xrow[:])

            # C used in pass 0 (echo back for debugging)
            nc.sync.dma_start(crow_out[0:1, :], c0row[:])

            # accumulators + margins out
            nc.sync.dma_start(acc_out[0, :, :], acc_spk[:])
            nc.sync.dma_start(acc_out[1, :, :], acc_mem[:])
            nc.sync.dma_start(acc_out[2, :, :], acc_g[:])
            if track_margin:
                nc.sync.dma_start(mm_out[:, :], mm[:])
            else:
                nc.sync.dma_start(mm_out[:, :], betat[:])

    nc.compile()
    _BUILD_CACHE[key] = nc
    return nc


# --------------------------------------------------------------- host logic
def _ema_chain_from_sums(sums, base, bias):
    """Replicate the reference's scalar f32 EMA/homeo chain from global
    per-step spike sums (exact integers).  Returns (C[t] used AT step t,
    final ema)."""
    ema = np.float32(0.0)
    C = np.zeros(T, np.float32)
    for t in range(T):
        homeo = np.float32(ADAPT_STRENGTH * max(np.float32(ema - HOMEO_RATE),
                                                np.float32(0.0)))
        C[t] = np.float32(np.float32(base + homeo) + bias)
        mean = np.float32(np.float32(sums[t]) * np.float32(1.0 / N_GLOB))
        ema = np.float32(math.fma(float(np.float32(0.99)), float(ema),
                                  float(np.float32(np.float32(0.01) * mean))))
    return C, ema


def _estimate_c0(input_current, base, bias, stride=16):
    """Cheap numpy f32 simulation of a strided neuron subsample with a local
    EMA to estimate the C[t] threshold-offset sequence."""
    x = np.ascontiguousarray(
        input_current[:, ::stride, :], dtype=np.float32)  # [B, F/stride, T]
    nb, nf, nt = x.shape
    x = x.reshape(nb * nf, nt)
    syn = np.zeros(nb * nf, np.float32)
    mem = np.zeros(nb * nf, np.float32)
    adp = np.zeros(nb * nf, np.float32)
    ema = np.float32(0.0)
    C = np.zeros(nt, np.float32)
    for t in range(nt):
        syn = BETA * syn + x[:, t]
        mem = ALPHA * mem + syn
        homeo = np.float32(ADAPT_STRENGTH * max(np.float32(ema - HOMEO_RATE),
                                                np.float32(0.0)))
        adp = GAMMA * adp + C1G * mem
        C[t] = np.float32(np.float32(base + homeo) + bias)
        thr = C[t] + ADAPT_STRENGTH * adp
        spike = (mem >= thr)
        ema = np.float32(0.99 * ema + 0.01 * np.float32(spike.mean()))
        mem = np.where(spike, mem - thr, mem).astype(np.float32)
    return C


def _sim_neurons(x_rows, C):
    """Bit-exact numpy replica of the device per-neuron chain for a set of
    neurons.  x_rows: [n, T] f32.  C: [T] f32 (positive C sequence)."""
    n = x_rows.shape[0]
    syn = np.zeros(n, np.float32)
    mem = np.zeros(n, np.float32)
    gst = np.zeros(n, np.float32)
    spikes = np.zeros((n, T), np.float32)
    mem_pre = np.zeros((n, T), np.float32)
    g_tr = np.zeros((n, T), np.float32)
    for t in range(T):
        # gpsimd: syn = (syn*beta); syn = syn + x  (two roundings)
        syn = (syn * BETA).astype(np.float32)
        syn = (syn + x_rows[:, t]).astype(np.float32)
        # V stt: mem = (mem*alpha) + syn
        mem = ((mem * ALPHA).astype(np.float32) + syn).astype(np.float32)
        mem_pre[:, t] = mem
        # V stt: u = (mem*k1) - G
        u = ((mem * K1).astype(np.float32) - gst).astype(np.float32)
        # ACT: tmp = (k2/k1)*u ; V stt: G = (G*g2) + tmp
        tmp = (u * KK).astype(np.float32)
        gst = ((gst * G2).astype(np.float32) + tmp).astype(np.float32)
        g_tr[:, t] = gst
        # spike = u >= C[t]
        sp = u >= C[t]
        spikes[:, t] = sp
        # e = relu(u - C[t]) ; mem = spike ? e : mem
        ev = np.maximum((u - C[t]).astype(np.float32), np.float32(0.0))
        mem = np.where(sp, ev, mem).astype(np.float32)
    return {"spikes": spikes, "mem_pre": mem_pre, "g": g_tr,
            "mem_post_last": mem}


def _shard_input(input_current):
    """[B, F, T] -> per-core [TB, P, G*FD] time-major tiles."""
    xs = []
    for k in range(N_CORES):
        xk = input_current[:, k * F_LOC:(k + 1) * F_LOC, :]  # [B, F_LOC, T]
        xk = np.ascontiguousarray(np.moveaxis(xk, -1, 0)).reshape(T, N_LOC)
        xk = xk.reshape(TB, G, P, FD).transpose(0, 2, 1, 3)
        xs.append(np.ascontiguousarray(xk.reshape(TB, P, G * FD),
                                       dtype=np.float32))
    return xs


def _unshard_spikes(spk_cores):
    """per-core [TB, P, G*FD] -> [B, F, T] float32."""
    out = np.empty((B, F, T), np.float32)
    for k, sk in enumerate(spk_cores):
        sk = sk.reshape(TB, P, G, FD).transpose(0, 2, 1, 3).reshape(T, N_LOC)
        sk = np.moveaxis(sk.reshape(T, B, F_LOC), 0, -1)
        out[:, k * F_LOC:(k + 1) * F_LOC, :] = sk
    return out


def _run_device(input_current, c0, base, bias, n_passes):
    from concourse.bass_utils import run_bass_kernel_spmd

    xs = _shard_input(input_current)
    nc = _build_kernel(n_passes, float(base), float(bias))
    in_maps = [{"x": xs[k], "c0": c0.reshape(1, T)} for k in range(N_CORES)]
    res = run_bass_kernel_spmd(nc, in_maps, core_ids=list(range(N_CORES)))
    return res.results


def kernel(input_current, threshold_scale, adaptation_bias):
    input_current = np.asarray(input_current, np.float32)
    threshold_scale = np.asarray(threshold_scale, np.float32)
    adaptation_bias = np.asarray(adaptation_bias, np.float32)
    assert input_current.shape == (B, F, T)

    scale_v = np.float32(threshold_scale.reshape(-1)[0])
    bias_v = np.float32(adaptation_bias.reshape(-1)[0])
    base_v = np.float32(THRESHOLD_BASE * scale_v)

    c0 = _estimate_c0(input_current, base_v, bias_v)

    if N_PASSES != 1:
        return _kernel_multipass(input_current, c0, base_v, bias_v)

    for attempt in range(3):
        r = _run_device(input_current, c0, base_v, bias_v, 1)
        out = _finish_single(input_current, c0, base_v, bias_v, r)
        if out is not None:
            return out
        # margin set too large -> c0 was far off; retry with corrected C
        acc = np.stack([rk["acc"] for rk in r])
        s_spk = acc.astype(np.float64).sum(axis=(0, 2))[0]
        c0, _ = _ema_chain_from_sums(s_spk, base_v, bias_v)
    raise RuntimeError("margin correction failed to converge")


def _finish_single(input_current, c0, base_v, bias_v, r):
    acc = np.stack([rk["acc"] for rk in r])       # [8, 3, P, T]
    g_sums = acc.astype(np.float64).sum(axis=(0, 2))
    s_spk, s_mem, s_g = g_sums
    mm = np.stack([rk["mm"] for rk in r])          # [8, P, FD]
    mm_flat = mm.reshape(N_CORES, N_LOC)           # neuron n = p*FD + j

    # candidate corrected C chain
    C1, _ = _ema_chain_from_sums(s_spk, base_v, bias_v)
    delta = float(np.abs(C1.astype(np.float64) - c0.astype(np.float64)).max())
    theta = 6.0 * delta + 1e-6
    if theta > THETA0:
        return None  # tracked margin radius insufficient; retry

    sel = mm_flat < np.float32(T)                  # [8, N_LOC] bool
    n_sel = int(sel.sum())
    if n_sel > MARGIN_CAP:
        return None

    # gather x rows for selected neurons
    core_idx, loc_idx = np.nonzero(sel)
    p_idx = loc_idx // FD
    j_idx = loc_idx % FD
    b_idx = loc_idx // F_LOC
    f_idx = core_idx * F_LOC + (loc_idx % F_LOC)
    x_rows = np.ascontiguousarray(
        input_current[b_idx, f_idx, :], dtype=np.float32)  # [n_sel, T]

    # device-replica sim under c0: must reproduce the device bitwise
    sim0 = _sim_neurons(x_rows, c0)

    # verify bit-exact replication against the device's spike output
    spk_cores = [r[k]["spk"] for k in range(N_CORES)]
    tbv, gv = np.divmod(np.arange(T), G)
    dev_rows = np.zeros((n_sel, T), np.float32)
    for i in range(n_sel):
        sk = spk_cores[core_idx[i]]
        dev_rows[i] = sk[tbv, p_idx[i], gv * FD + j_idx[i]]
    mismatch = int((dev_rows != sim0["spikes"]).sum())
    if mismatch:
        sys.stderr.write(
            f"kernel.py: host/device replica mismatch ({mismatch} elems), "
            "falling back to 2-pass device mode\n")
        return _kernel_multipass(input_current, c0, base_v, bias_v)

    # iterate the tiny fixed point on the selected set
    base_spk = s_spk - sim0["spikes"].astype(np.float64).sum(axis=0)
    base_mem = s_mem - sim0["mem_pre"].astype(np.float64).sum(axis=0)
    base_g = s_g - sim0["g"].astype(np.float64).sum(axis=0)
    xtra0 = float(sum(rk["xtra"][0, 0] for rk in r))
    base_mlast = xtra0 - float(sim0["mem_post_last"].astype(np.float64).sum())

    C_cur = C1
    sim = sim0
    converged = False
    for _ in range(12):
        sim = _sim_neurons(x_rows, C_cur)
        s_spk_new = base_spk + sim["spikes"].astype(np.float64).sum(axis=0)
        C_next, _ = _ema_chain_from_sums(s_spk_new, base_v, bias_v)
        if np.array_equal(C_next.view(np.uint32), C_cur.view(np.uint32)):
            converged = True
            break
        C_cur = C_next
    if not converged:
        return None

    # safety: all excursions must stay inside the margin set
    exc = float(np.abs(C_cur.astype(np.float64) -
                       c0.astype(np.float64)).max())
    if exc > theta / 2.0:
        return None

    s_spk = base_spk + sim["spikes"].astype(np.float64).sum(axis=0)
    s_mem = base_mem + sim["mem_pre"].astype(np.float64).sum(axis=0)
    s_g = base_g + sim["g"].astype(np.float64).sum(axis=0)
    mlast = base_mlast + float(sim["mem_post_last"].astype(np.float64).sum())
    _, ema_final = _ema_chain_from_sums(s_spk, base_v, bias_v)

    # patch spikes of the selected neurons
    spikes = _unshard_spikes(spk_cores)
    spikes[b_idx, f_idx, :] = sim["spikes"]

    return _assemble(input_current, spikes, s_spk, s_mem, s_g, mlast,
                     C_cur, ema_final)


def _assemble(input_current, spikes, s_spk, s_mem, s_g, mem_last_sum,
              C_used, ema_final):
    # syn sums reconstructed from input sums (f64 model of the f32 chain)
    sx = input_current.astype(np.float64).sum(axis=(0, 1))  # [T]
    s_syn = np.zeros(T, np.float64)
    run = 0.0
    for t in range(T):
        run = float(BETA) * run + sx[t]
        s_syn[t] = run

    mem_post = np.empty(T, np.float64)
    mem_post[:T - 1] = (s_mem[1:] - s_syn[1:]) / np.float64(ALPHA)
    mem_post[T - 1] = mem_last_sum
    mem_trace = (mem_post / N_GLOB).astype(np.float32)

    thr_trace = (C_used.astype(np.float64)
                 + s_g / (np.float64(GAMMA) * N_GLOB)).astype(np.float32)

    adaptation_mean = np.float32(
        s_g[T - 1] / (0.1 * np.float64(GAMMA)) / N_GLOB)

    return (spikes, mem_trace, thr_trace, np.float32(ema_final),
            adaptation_mean)


def _kernel_multipass(input_current, c0, base_v, bias_v):
    r = _run_device(input_current, c0, base_v, bias_v, 2)
    acc = np.stack([rk["acc"] for rk in r])
    g_sums = acc.astype(np.float64).sum(axis=(0, 2))
    s_spk, s_mem, s_g = g_sums
    c_dev = r[0]["crow"][1].astype(np.float32)
    _, ema_final = _ema_chain_from_sums(s_spk, base_v, bias_v)
    mlast = float(sum(rk["xtra"][0, 0] for rk in r))
    spikes = _unshard_spikes([rk["spk"] for rk in r])
    return _assemble(input_current, spikes, s_spk, s_mem, s_g, mlast,
                     c_dev, ema_final)


if __name__ == "__main__":
    rng = np.random.default_rng(0)
    x = rng.standard_normal((B, F, T)).astype(np.float32)
    t0 = time.time()
    out = kernel(x, np.ones(1, np.float32), np.zeros(1, np.float32))
    print("kernel wall:", time.time() - t0)
    for o in out:
        print(np.shape(o), np.asarray(o).dtype)


# revision 21
# speedup vs baseline: 1.2001x; 1.2001x over previous
"""Adaptive-threshold LIF neuron recurrence (SNN) on 8 Trainium2 NeuronCores.

Strategy
--------
The recurrence is data-parallel over the 262144 neurons except for a scalar
firing-rate EMA that couples every neuron at every timestep (the spike MEAN
feeds the next step's threshold).  A per-step AllReduce would cost ~10us x
128 steps, so instead:

  host:   estimate the threshold-offset sequence C0[t] from a neuron
          subsample (cheap numpy sim)
  device: ONE data-parallel pass with C0 per core, recording per-step
          spike/membrane/adaptation partial sums (exact f32 integers for
          spikes) AND each neuron's minimum |u - C0| margin over time
  host:   compute the exact global EMA chain from the (integer) spike sums
          -> corrected C[t]; every neuron whose min margin exceeds the C
          correction provably has an identical trajectory; the few
          (~10^2-10^3) neurons inside the margin are re-simulated in numpy
          with bit-exact replicas of the device arithmetic, iterating the
          tiny fixed point until the C chain is stable; spikes and sums are
          patched accordingly.

Spike sums are sums of 0/1 values, so every f32 reduction of them is exact
(integers < 2^24), which makes the host EMA chain exact.  A 2-pass fully
on-device variant (P1 -> AllReduce -> P2) is kept as a fallback.

Per-core layout: features sharded 8 ways (1024 per core), time-major tiles
x[tb, p, g*256+j] = input[t = tb*G+g, neuron n = p*256+j], n = b*1024+f.

Per-step device math (one [128,256] f32 tile per core per step), with
G := 0.1*gamma*adapt pre-scaled so the threshold compare needs no extra op:
  syn   = (syn * beta) + x_t                 (GPSIMD: 2x tensor_tensor)
  mem   = (mem * alpha) + syn                (V: stt, accum -> mem sums)
  u     = (mem * k1) - G                     (V: stt)   [u == mem - 0.1*adapt]
  tmp   = k2 * mem                           (ACT copy)
  G     = (G * gamma) + tmp                  (V: stt, accum -> G sums)
  spike = u >= C_t                           (V: tensor_scalar, accum -> sums)
  e     = relu(u - C_t)                      (ACT relu, bias = -C)
  mem   = spike ? e : mem                    (V: copy_predicated)
  ab    = |u - C_t|                          (ACT abs, bias = -C)
  sg    = sign(ab - theta0)                  (ACT sign: -1/0 if near margin)
  asum  = asum + sg                          (GPSIMD tensor_tensor add)
(neuron is provably unaffected by C refinements iff asum == T)
"""

import math
import sys
import time

if "/opt/trn_rl_repo" not in sys.path:
    sys.path.insert(0, "/opt/trn_rl_repo")

import numpy as np

# ---------------------------------------------------------------- constants
B, F, T = 32, 8192, 128
N_CORES = 8
F_LOC = F // N_CORES            # 1024 features per core
N_LOC = B * F_LOC               # 32768 neurons per core
N_GLOB = B * F                  # 262144 neurons total
P = 128                         # SBUF partitions
FD = N_LOC // P                 # 256 free-dim elems per step tile
G = 16                          # timesteps per DMA group
TB = T // G                     # 8 groups

# exact f32 constants as produced by the jax reference (verified bitwise)
ALPHA = np.uint32(0x3F7383C5).view(np.float32)  # exp(-.001/.02)
BETA = np.uint32(0x3F519857).view(np.float32)   # exp(-.001/.005)
GAMMA = np.uint32(0x3F7D73E8).view(np.float32)  # exp(-.001/.1)
C1G = np.uint32(0x3C230600).view(np.float32)    # 1 - gamma
ADAPT_STRENGTH = np.float32(0.1)
HOMEO_RATE = np.float32(0.01)
THRESHOLD_BASE = np.float32(1.0)

K1 = np.float32(1.0 - 0.1 * float(C1G))            # 1 - 0.1*(1-gamma)
K2 = np.float32(0.1 * float(GAMMA) * float(C1G))   # 0.1*gamma*(1-gamma)
# G update rewritten in terms of u (so no engine reads mem after the reset):
#   G_t = (gamma + k2/k1)*G_{t-1} + (k2/k1)*u_t
KK = np.float32(float(K2) / float(K1))             # k2/k1
G2 = np.float32(float(GAMMA) + float(K2) / float(K1))

# per-step beta powers (f32) for the synaptic prescaling trick:
# host feeds x'_t = x_t / beta^t, device computes S_t = S_{t-1} + x'_t and
# syn_t = beta^t * S_t (one ACT op), saving a GPSIMD multiply per step
BP32 = np.float32(np.float64(BETA)) ** 0  # placeholder, replaced below
BP32 = np.array([np.float32(np.float64(BETA) ** t) for t in range(T)],
                dtype=np.float32)

N_PASSES = 1          # 1 = single pass + host margin correction (default)
MARGIN_CAP = 40000    # fall back to a device re-run above this many neurons
THETA0 = 3e-4         # margin radius tracked on device (abs units of u)

_BUILD_CACHE = {}


# ------------------------------------------------------------- device build
def _build_kernel(n_passes=N_PASSES, base=1.0, bias=0.0):
    key = (n_passes, float(base), float(bias))
    if key in _BUILD_CACHE:
        return _BUILD_CACHE[key]

    import concourse.bacc as bacc
    import concourse.mybir as mybir
    from concourse import tile

    DT = mybir.dt.float32
    AF = mybir.ActivationFunctionType
    OP = mybir.AluOpType
    I32 = mybir.dt.int32

    nc = bacc.Bacc(None, target_bir_lowering=False, debug=False,
                   num_devices=N_CORES)

    x_in = nc.dram_tensor("x", [TB, P, G * FD], DT, kind="ExternalInput")
    c0_in = nc.dram_tensor("c0", [1, T], DT, kind="ExternalInput")

    spk_out = nc.dram_tensor("spk", [TB, P, G * FD], DT, kind="ExternalOutput")
    acc_out = nc.dram_tensor("acc", [3, P, T], DT, kind="ExternalOutput")
    crow_out = nc.dram_tensor("crow", [max(n_passes, 2), T], DT,
                              kind="ExternalOutput")
    xtra_out = nc.dram_tensor("xtra", [1, 8], DT, kind="ExternalOutput")
    mm_out = nc.dram_tensor("mm", [P, FD], DT, kind="ExternalOutput")

    with tile.TileContext(nc) as tc:
        with (
            tc.tile_pool(name="state", bufs=1) as st,
            tc.tile_pool(name="xload", bufs=3) as xl,
            tc.tile_pool(name="sout", bufs=2) as so,
            tc.tile_pool(name="psum", bufs=2, space="PSUM") as ps,
            tc.tile_pool(name="dram", bufs=1, space="DRAM") as dram,
        ):
            mem = st.tile([P, FD], DT, tag="mem")
            gst = st.tile([P, FD], DT, tag="gst")
            mm = st.tile([P, FD], DT, tag="mm")
            spk_s = st.tile([P, FD], DT, tag="spk_s")
            ones = st.tile([P, 1], DT, tag="ones")
            nc.vector.memset(ones[:], 1.0)

            nth0 = st.tile([P, 1], DT, tag="nth0")
            nc.vector.memset(nth0[:], -float(THETA0))
            track_margin = n_passes == 1
            if track_margin:
                nc.gpsimd.memset(mm[:], 0.0)

            # per-pass C matrices ([128, T] broadcast across partitions):
            # positive C for the spike compare (tensor_scalar is_ge),
            # negated C as the ACT Relu/Abs bias
            cmat = [
                st.tile([P, T], DT, tag=f"cmat{p}", name=f"cmat{p}")
                for p in range(n_passes)
            ]
            nmat = [
                st.tile([P, T], DT, tag=f"nmat{p}", name=f"nmat{p}")
                for p in range(n_passes)
            ]
            c0row = st.tile([1, T], DT, tag="c0row")
            nc.sync.dma_start(c0row[:], c0_in[:, :])
            nc0row = st.tile([1, T], DT, tag="nc0row")
            nc.vector.tensor_scalar(nc0row[:], c0row[:], -1.0, None, OP.mult)
            nc.gpsimd.partition_broadcast(cmat[0][:], c0row[:], channels=P)
            nc.gpsimd.partition_broadcast(nmat[0][:], nc0row[:], channels=P)

            # accumulators: [128, T] per quantity (last pass overwrites)
            acc_spk = st.tile([P, T], DT, tag="acc_spk")
            acc_mem = st.tile([P, T], DT, tag="acc_mem")
            acc_g = st.tile([P, T], DT, tag="acc_g")

            # constant row of 0.99 for the ema scan (2-pass mode)
            row99 = st.tile([1, T], DT, tag="row99")
            nc.vector.memset(row99[:], 0.99)

            for p_i in range(n_passes):
                last = p_i == n_passes - 1
                C = cmat[p_i]
                NC = nmat[p_i]
                syn = st.tile([P, FD], DT, tag="syn", name="syn0", bufs=2)
                nc.gpsimd.memset(syn[:], 0.0)
                nc.vector.memset(mem[:], 0.0)
                nc.vector.memset(gst[:], 0.0)

                for tb in range(TB):
                    if tb == 0:
                        # split the first load so compute starts early
                        xt_a = xl.tile([P, 4 * FD], DT, tag="xta",
                                       name="xta")
                        nc.sync.dma_start(xt_a[:], x_in[0, :, 0:4 * FD])
                        xt_b = xl.tile([P, 12 * FD], DT, tag="xtb",
                                       name="xtb")
                        nc.sync.dma_start(xt_b[:], x_in[0, :, 4 * FD:])
                    else:
                        xt = xl.tile([P, G * FD], DT, tag="xt", name="xt")
                        nc.sync.dma_start(xt[:], x_in[tb, :, :])
                    if last:
                        ot = so.tile([P, G * FD], DT, tag="ot")
                    for g in range(G):
                        t = tb * G + g
                        if tb == 0:
                            if g < 4:
                                xs = xt_a[:, g * FD:(g + 1) * FD]
                            else:
                                xs = xt_b[:, (g - 4) * FD:(g - 3) * FD]
                        else:
                            xs = xt[:, g * FD:(g + 1) * FD]
                        u = st.tile([P, FD], DT, tag="u", name="u", bufs=2)
                        e = st.tile([P, FD], DT, tag="e", name="e", bufs=2)
                        tmp = st.tile([P, FD], DT, tag="tmp", name="tmp",
                                      bufs=2)
                        ab = st.tile([P, FD], DT, tag="ab", name="ab",
                                     bufs=2)
                        sg = st.tile([P, FD], DT, tag="sg", name="sg",
                                     bufs=2)
                        # S_t = S_{t-1} + x'_t  (GPSIMD, double-buffered);
                        # syn_t = beta^t * S_t   (ACT, immediate scale)
                        syn_new = st.tile([P, FD], DT, tag="syn",
                                          name="syn", bufs=2)
                        nc.gpsimd.tensor_tensor(syn_new[:], syn[:], xs,
                                                OP.add)
                        syn = syn_new
                        synu = st.tile([P, FD], DT, tag="synu", name="synu",
                                       bufs=2)
                        nc.scalar.activation(synu[:], syn[:], AF.Copy,
                                             scale=float(BP32[t]))
                        nc.vector.scalar_tensor_tensor(
                            mem[:], mem[:], float(ALPHA), synu[:],
                            OP.mult, OP.add,
                            accum_out=acc_mem[:, t:t + 1] if last else None,
                        )
                        nc.vector.scalar_tensor_tensor(
                            u[:], mem[:], float(K1), gst[:],
                            OP.mult, OP.subtract,
                        )
                        # reset value e = relu(u + (-C)) on the ACT engine
                        # (emitted early: it is on the critical V loop)
                        nc.scalar.activation(
                            e[:], u[:], AF.Relu, bias=NC[:, t:t + 1],
                            scale=1.0,
                        )
                        if last:
                            sp_ap = ot[:, g * FD:(g + 1) * FD]
                        else:
                            sp_ap = spk_s[:]
                        # spike = (u >= C), accum(reduce add) -> step sums
                        nc.vector.tensor_scalar(
                            sp_ap, u[:], C[:, t:t + 1], 0.0,
                            OP.is_ge, OP.add,
                            accum_out=acc_spk[:, t:t + 1],
                        )
                        nc.vector.copy_predicated(
                            mem[:], sp_ap.bitcast(I32), e[:]
                        )
                        nc.scalar.activation(tmp[:], u[:], AF.Copy,
                                             scale=float(KK))
                        nc.vector.scalar_tensor_tensor(
                            gst[:], gst[:], float(G2), tmp[:],
                            OP.mult, OP.add,
                            accum_out=acc_g[:, t:t + 1] if last else None,
                        )
                        if track_margin:
                            nc.scalar.activation(
                                ab[:], u[:], AF.Abs, bias=NC[:, t:t + 1],
                                scale=1.0,
                            )
                            nc.scalar.activation(
                                sg[:], ab[:], AF.Sign, bias=nth0[:, 0:1],
                                scale=1.0,
                            )
                            nc.gpsimd.tensor_tensor(mm[:], mm[:], sg[:],
                                                    OP.add)
                    if last:
                        nc.sync.dma_start(spk_out[tb, :, :], ot[:])

                # ---- inter-pass: global spike sums -> next C (2-pass mode)
                if not last:
                    pt = ps.tile([1, T], DT, tag="pt")
                    nc.tensor.matmul(pt[:], ones[:], acc_spk[:],
                                     start=True, stop=True)
                    srow = st.tile([1, T], DT, tag=f"srow{p_i}",
                                   name=f"srow{p_i}")
                    nc.scalar.copy(srow[:], pt[:])
                    cc_in = dram.tile([1, T], DT, name=f"ccin{p_i}")
                    cc_out = dram.tile([1, T], DT, name=f"ccout{p_i}")
                    nc.sync.dma_start(cc_in[:], srow[:])
                    nc.gpsimd.collective_compute(
                        "AllReduce", OP.add,
                        replica_groups=[list(range(N_CORES))],
                        ins=[cc_in.opt()], outs=[cc_out.opt()],
                    )
                    sglob = st.tile([1, T], DT, tag=f"sglob{p_i}",
                                    name=f"sglob{p_i}")
                    nc.sync.dma_start(sglob[:], cc_out[:])
                    # ema chain: msc = sums * (0.01/N_GLOB); scan; homeo; C
                    mean_scale = np.float32(
                        np.float32(0.01) * np.float32(1.0 / N_GLOB))
                    msc = st.tile([1, T], DT, tag=f"msc{p_i}",
                                  name=f"msc{p_i}")
                    nc.vector.tensor_scalar(
                        msc[:], sglob[:], float(mean_scale), None, OP.mult)
                    ema = st.tile([1, T], DT, tag=f"ema{p_i}",
                                  name=f"ema{p_i}")
                    nc.vector.tensor_tensor_scan(
                        ema[:], row99[:], msc[:], 0.0, OP.mult, OP.add)
                    hm = st.tile([1, T], DT, tag=f"hm{p_i}", name=f"hm{p_i}")
                    nc.vector.tensor_scalar(
                        hm[:], ema[:], float(HOMEO_RATE), 0.0,
                        OP.subtract, OP.max)
                    cn = st.tile([1, T], DT, tag=f"cn{p_i}", name=f"cn{p_i}")
                    # C[0] is pass-independent (ema starts at 0)
                    nc.vector.tensor_copy(cn[:, 0:1], c0row[:, 0:1])
                    nc.vector.tensor_scalar(
                        cn[:, 1:T], hm[:, 0:T - 1], float(ADAPT_STRENGTH),
                        float(base), OP.mult, OP.add)
                    if float(bias) != 0.0:
                        nc.vector.tensor_scalar(
                            cn[:, 1:T], cn[:, 1:T], float(bias), None,
                            OP.add)
                    ncn = st.tile([1, T], DT, tag=f"ncn{p_i}",
                                  name=f"ncn{p_i}")
                    nc.vector.tensor_scalar(
                        ncn[:], cn[:], -1.0, None, OP.mult)
                    nc.gpsimd.partition_broadcast(
                        cmat[p_i + 1][:], cn[:], channels=P)
                    nc.gpsimd.partition_broadcast(
                        nmat[p_i + 1][:], ncn[:], channels=P)
                    nc.sync.dma_start(crow_out[p_i + 1:p_i + 2, :], cn[:])

            # ---- final extras: sum of post-reset mem at t=T-1
            memsum = st.tile([P, 1], DT, tag="memsum")
            nc.vector.reduce_sum(memsum[:], mem[:], axis=mybir.AxisListType.X)
            ptx = ps.tile([1, 1], DT, tag="ptx")
            nc.tensor.matmul(ptx[:], ones[:], memsum[:], start=True, stop=True)
            xrow = st.tile([1, 8], DT, tag="xrow")
            nc.vector.memset(xrow[:], 0.0)
            nc.scalar.copy(xrow[:, 0:1], ptx[:])
            nc.sync.dma_start(xtra_out[:, :], xrow[:])

            # C used in pass 0 (echo back for debugging)
            nc.sync.dma_start(crow_out[0:1, :], c0row[:])

            # accumulators + margins out
            nc.sync.dma_start(acc_out[0, :, :], acc_spk[:])
            nc.sync.dma_start(acc_out[1, :, :], acc_mem[:])
            nc.sync.dma_start(acc_out[2, :, :], acc_g[:])
            if track_margin:
                nc.sync.dma_start(mm_out[:, :], mm[:])
            else:
                nc.sync.dma_start(mm_out[:, :], betat[:])

    nc.compile()
    _BUILD_CACHE[key] = nc
    return nc


# --------------------------------------------------------------- host logic
def _ema_chain_from_sums(sums, base, bias):
    """Replicate the reference's scalar f32 EMA/homeo chain from global
    per-step spike sums (exact integers).  Returns (C[t] used AT step t,
    final ema)."""
    ema = np.float32(0.0)
    C = np.zeros(T, np.float32)
    for t in range(T):
        homeo = np.float32(ADAPT_STRENGTH * max(np.float32(ema - HOMEO_RATE),
                                                np.float32(0.0)))
        C[t] = np.float32(np.float32(base + homeo) + bias)
        mean = np.float32(np.float32(sums[t]) * np.float32(1.0 / N_GLOB))
        ema = np.float32(math.fma(float(np.float32(0.99)), float(ema),
                                  float(np.float32(np.float32(0.01) * mean))))
    return C, ema


def _estimate_c0(input_current, base, bias, stride=16):
    """Cheap numpy f32 simulation of a strided neuron subsample with a local
    EMA to estimate the C[t] threshold-offset sequence."""
    x = np.ascontiguousarray(
        input_current[:, ::stride, :], dtype=np.float32)  # [B, F/stride, T]
    nb, nf, nt = x.shape
    x = x.reshape(nb * nf, nt)
    syn = np.zeros(nb * nf, np.float32)
    mem = np.zeros(nb * nf, np.float32)
    adp = np.zeros(nb * nf, np.float32)
    ema = np.float32(0.0)
    C = np.zeros(nt, np.float32)
    for t in range(nt):
        syn = BETA * syn + x[:, t]
        mem = ALPHA * mem + syn
        homeo = np.float32(ADAPT_STRENGTH * max(np.float32(ema - HOMEO_RATE),
                                                np.float32(0.0)))
        adp = GAMMA * adp + C1G * mem
        C[t] = np.float32(np.float32(base + homeo) + bias)
        thr = C[t] + ADAPT_STRENGTH * adp
        spike = (mem >= thr)
        ema = np.float32(0.99 * ema + 0.01 * np.float32(spike.mean()))
        mem = np.where(spike, mem - thr, mem).astype(np.float32)
    return C


def _sim_neurons(x_rows, C):
    """Bit-exact numpy replica of the device per-neuron chain for a set of
    neurons.  x_rows: [n, T] f32.  C: [T] f32 (positive C sequence)."""
    n = x_rows.shape[0]
    syn = np.zeros(n, np.float32)
    mem = np.zeros(n, np.float32)
    gst = np.zeros(n, np.float32)
    spikes = np.zeros((n, T), np.float32)
    mem_pre = np.zeros((n, T), np.float32)
    g_tr = np.zeros((n, T), np.float32)
    for t in range(T):
        # gpsimd: S += x' ; ACT: syn = beta^t * S
        syn = (syn + x_rows[:, t]).astype(np.float32)
        synu = (syn * BP32[t]).astype(np.float32)
        # V stt: mem = (mem*alpha) + synu
        mem = ((mem * ALPHA).astype(np.float32) + synu).astype(np.float32)
        mem_pre[:, t] = mem
        # V stt: u = (mem*k1) - G
        u = ((mem * K1).astype(np.float32) - gst).astype(np.float32)
        # ACT: tmp = (k2/k1)*u ; V stt: G = (G*g2) + tmp
        tmp = (u * KK).astype(np.float32)
        gst = ((gst * G2).astype(np.float32) + tmp).astype(np.float32)
        g_tr[:, t] = gst
        # spike = u >= C[t]
        sp = u >= C[t]
        spikes[:, t] = sp
        # e = relu(u - C[t]) ; mem = spike ? e : mem
        ev = np.maximum((u - C[t]).astype(np.float32), np.float32(0.0))
        mem = np.where(sp, ev, mem).astype(np.float32)
    return {"spikes": spikes, "mem_pre": mem_pre, "g": g_tr,
            "mem_post_last": mem}


IB64 = 1.0 / BP32.astype(np.float64)        # exact per-step inverse scales


def _prescale(input_current):
    """x'_[b,f,t] = f32(x / beta^t) plus its per-step global sums."""
    x64 = input_current.astype(np.float64) * IB64[None, None, :]
    xp = x64.astype(np.float32)
    sxp = x64.sum(axis=(0, 1))                 # [T] f64 sums of x'
    return xp, sxp


def _shard_input(xp):
    """prescaled [B, F, T] -> per-core [TB, P, G*FD] time-major tiles."""
    xs = []
    for k in range(N_CORES):
        xk = xp[:, k * F_LOC:(k + 1) * F_LOC, :]  # [B, F_LOC, T]
        xk = np.ascontiguousarray(np.moveaxis(xk, -1, 0)).reshape(T, N_LOC)
        xk = xk.reshape(TB, G, P, FD).transpose(0, 2, 1, 3)
        xs.append(np.ascontiguousarray(xk.reshape(TB, P, G * FD),
                                       dtype=np.float32))
    return xs


def _unshard_spikes(spk_cores):
    """per-core [TB, P, G*FD] -> [B, F, T] float32."""
    out = np.empty((B, F, T), np.float32)
    for k, sk in enumerate(spk_cores):
        sk = sk.reshape(TB, P, G, FD).transpose(0, 2, 1, 3).reshape(T, N_LOC)
        sk = np.moveaxis(sk.reshape(T, B, F_LOC), 0, -1)
        out[:, k * F_LOC:(k + 1) * F_LOC, :] = sk
    return out


def _run_device(xs, c0, base, bias, n_passes):
    from concourse.bass_utils import run_bass_kernel_spmd

    nc = _build_kernel(n_passes, float(base), float(bias))
    in_maps = [{"x": xs[k], "c0": c0.reshape(1, T)} for k in range(N_CORES)]
    res = run_bass_kernel_spmd(nc, in_maps, core_ids=list(range(N_CORES)))
    return res.results


def kernel(input_current, threshold_scale, adaptation_bias):
    input_current = np.asarray(input_current, np.float32)
    threshold_scale = np.asarray(threshold_scale, np.float32)
    adaptation_bias = np.asarray(adaptation_bias, np.float32)
    assert input_current.shape == (B, F, T)

    scale_v = np.float32(threshold_scale.reshape(-1)[0])
    bias_v = np.float32(adaptation_bias.reshape(-1)[0])
    base_v = np.float32(THRESHOLD_BASE * scale_v)

    c0 = _estimate_c0(input_current, base_v, bias_v)
    xp, sxp = _prescale(input_current)
    xs = _shard_input(xp)

    if N_PASSES != 1:
        return _kernel_multipass(xs, sxp, c0, base_v, bias_v)

    for attempt in range(3):
        r = _run_device(xs, c0, base_v, bias_v, 1)
        out = _finish_single(xp, sxp, c0, base_v, bias_v, r)
        if out is not None:
            return out
        # margin set too large -> c0 was far off; retry with corrected C
        acc = np.stack([rk["acc"] for rk in r])
        s_spk = acc.astype(np.float64).sum(axis=(0, 2))[0]
        c0, _ = _ema_chain_from_sums(s_spk, base_v, bias_v)
    # last resort: fully on-device 2-pass mode
    return _kernel_multipass(xs, sxp, c0, base_v, bias_v)


def _finish_single(xp, sxp, c0, base_v, bias_v, r):
    acc = np.stack([rk["acc"] for rk in r])       # [8, 3, P, T]
    g_sums = acc.astype(np.float64).sum(axis=(0, 2))
    s_spk, s_mem, s_g = g_sums
    mm = np.stack([rk["mm"] for rk in r])          # [8, P, FD]
    mm_flat = mm.reshape(N_CORES, N_LOC)           # neuron n = p*FD + j

    # candidate corrected C chain
    C1, _ = _ema_chain_from_sums(s_spk, base_v, bias_v)
    delta = float(np.abs(C1.astype(np.float64) - c0.astype(np.float64)).max())
    theta = 6.0 * delta + 1e-6
    if theta > THETA0:
        return None  # tracked margin radius insufficient; retry

    sel = mm_flat < np.float32(T)                  # [8, N_LOC] bool
    n_sel = int(sel.sum())
    if n_sel > MARGIN_CAP:
        return None

    # gather x rows for selected neurons
    core_idx, loc_idx = np.nonzero(sel)
    p_idx = loc_idx // FD
    j_idx = loc_idx % FD
    b_idx = loc_idx // F_LOC
    f_idx = core_idx * F_LOC + (loc_idx % F_LOC)
    x_rows = np.ascontiguousarray(
        xp[b_idx, f_idx, :], dtype=np.float32)  # [n_sel, T] (prescaled)

    # device-replica sim under c0: must reproduce the device bitwise
    sim0 = _sim_neurons(x_rows, c0)

    # verify bit-exact replication against the device's spike output
    spk_cores = [r[k]["spk"] for k in range(N_CORES)]
    tbv, gv = np.divmod(np.arange(T), G)
    dev_rows = np.zeros((n_sel, T), np.float32)
    for i in range(n_sel):
        sk = spk_cores[core_idx[i]]
        dev_rows[i] = sk[tbv, p_idx[i], gv * FD + j_idx[i]]
    mismatch = int((dev_rows != sim0["spikes"]).sum())
    if mismatch:
        sys.stderr.write(
            f"kernel.py: host/device replica mismatch ({mismatch} elems), "
            "falling back to 2-pass device mode\n")
        return None

    # iterate the tiny fixed point on the selected set
    base_spk = s_spk - sim0["spikes"].astype(np.float64).sum(axis=0)
    base_mem = s_mem - sim0["mem_pre"].astype(np.float64).sum(axis=0)
    base_g = s_g - sim0["g"].astype(np.float64).sum(axis=0)
    xtra0 = float(sum(rk["xtra"][0, 0] for rk in r))
    base_mlast = xtra0 - float(sim0["mem_post_last"].astype(np.float64).sum())

    C_cur = C1
    sim = sim0
    converged = False
    for _ in range(12):
        sim = _sim_neurons(x_rows, C_cur)
        s_spk_new = base_spk + sim["spikes"].astype(np.float64).sum(axis=0)
        C_next, _ = _ema_chain_from_sums(s_spk_new, base_v, bias_v)
        if np.array_equal(C_next.view(np.uint32), C_cur.view(np.uint32)):
            converged = True
            break
        C_cur = C_next
    if not converged:
        return None

    # safety: all excursions must stay inside the margin set
    exc = float(np.abs(C_cur.astype(np.float64) -
                       c0.astype(np.float64)).max())
    if exc > theta / 2.0:
        return None

    s_spk = base_spk + sim["spikes"].astype(np.float64).sum(axis=0)
    s_mem = base_mem + sim["mem_pre"].astype(np.float64).sum(axis=0)
    s_g = base_g + sim["g"].astype(np.float64).sum(axis=0)
    mlast = base_mlast + float(sim["mem_post_last"].astype(np.float64).sum())
    _, ema_final = _ema_chain_from_sums(s_spk, base_v, bias_v)

    # patch spikes of the selected neurons
    spikes = _unshard_spikes(spk_cores)
    spikes[b_idx, f_idx, :] = sim["spikes"]

    return _assemble(sxp, spikes, s_spk, s_mem, s_g, mlast,
                     C_cur, ema_final)


def _assemble(sxp, spikes, s_spk, s_mem, s_g, mem_last_sum,
              C_used, ema_final):
    # syn sums reconstructed from prescaled-input sums (f64 model):
    # S_t = S_{t-1} + sum(x'_t);  syn_t = beta^t * S_t
    s_syn = np.zeros(T, np.float64)
    run = 0.0
    for t in range(T):
        run = run + sxp[t]
        s_syn[t] = float(BP32[t]) * run

    mem_post = np.empty(T, np.float64)
    mem_post[:T - 1] = (s_mem[1:] - s_syn[1:]) / np.float64(ALPHA)
    mem_post[T - 1] = mem_last_sum
    mem_trace = (mem_post / N_GLOB).astype(np.float32)

    thr_trace = (C_used.astype(np.float64)
                 + s_g / (np.float64(GAMMA) * N_GLOB)).astype(np.float32)

    adaptation_mean = np.float32(
        s_g[T - 1] / (0.1 * np.float64(GAMMA)) / N_GLOB)

    return (spikes, mem_trace, thr_trace, np.float32(ema_final),
            adaptation_mean)


def _kernel_multipass(xs, sxp, c0, base_v, bias_v):
    r = _run_device(xs, c0, base_v, bias_v, 2)
    acc = np.stack([rk["acc"] for rk in r])
    g_sums = acc.astype(np.float64).sum(axis=(0, 2))
    s_spk, s_mem, s_g = g_sums
    c_dev = r[0]["crow"][1].astype(np.float32)
    _, ema_final = _ema_chain_from_sums(s_spk, base_v, bias_v)
    mlast = float(sum(rk["xtra"][0, 0] for rk in r))
    spikes = _unshard_spikes([rk["spk"] for rk in r])
    return _assemble(sxp, spikes, s_spk, s_mem, s_g, mlast,
                     c_dev, ema_final)


if __name__ == "__main__":
    rng = np.random.default_rng(0)
    x = rng.standard_normal((B, F, T)).astype(np.float32)
    t0 = time.time()
    out = kernel(x, np.ones(1, np.float32), np.zeros(1, np.float32))
    print("kernel wall:", time.time() - t0)
    for o in out:
        print(np.shape(o), np.asarray(o).dtype)
